# revision 4
# baseline (speedup 1.0000x reference)
"""GAT (nn_GAT_76536317214930) on 8 TRN2 NeuronCores.

The reference's attention softmax is dead code; each layer is
    emb = elu(adj @ (x @ Wcat))        with heads concatenated on features,
then out = elu(emb1) @ lin_w + lin_b and log_softmax.

Sharding: rows (destination nodes) of adj split across 8 cores. Each core
holds adjT shard [8192, 1024] (bf16, SBUF-resident, used by both layers),
computes H = x @ Wcat for its own rows, AllGathers H, then accumulates
P^T = H^T-chunks (stationary) @ adjT-chunks (moving) on the PE.

All feature-major [64, n] intermediates; node-major tiles are produced by
matmuls with the feature-major tensor as lhsT (contracting over features),
so no explicit transposes are needed anywhere.
"""
import numpy as np
import ml_dtypes

import concourse.bass as bass
import concourse.bacc as bacc
import concourse.mybir as mybir
import concourse.tile as tile
from concourse.bass_utils import run_bass_kernel_spmd

NCORES = 8
N = 8192          # nodes
NFEAT = 512       # input features
F = 64            # NHEADS * NHID = 4*16
NCLASS = 40
SH = N // NCORES  # 1024 rows per core
NCH = N // 128    # 64 contraction chunks (global nodes)
SHC = SH // 128   # 8 node tiles per core shard
XCH = NFEAT // 128  # 4 chunks of input features

BF16 = mybir.dt.bfloat16
F32 = mybir.dt.float32
AF = mybir.ActivationFunctionType
ALU = mybir.AluOpType

ADJ_DMA_GROUPS = 16  # adjT loaded in 16 DMAs of 4 chunks (1 MiB each)

_NC_CACHE = {}


def _emit_elu(nc, pool, out_f32, out_alt, src_psum_halves, tmp_tag):
    """elu(x) = max(x, exp(min(x, 0)) - 1), per [64, 512] half.

    src_psum_halves: list of (psum_ap, col_offset) covering [64, 1024].
    out_f32: [64, 1024] f32 SBUF AP or None.
    out_alt: (ap, dtype-done-by-ap) secondary output [64, 1024] or None.
    """
    for ps, off in src_psum_halves:
        w = ps.shape[-1]
        m_t = pool.tile([64, w], F32, name=f"{tmp_tag}_m", tag="elu_m", bufs=2)
        e_t = pool.tile([64, w], F32, name=f"{tmp_tag}_e", tag="elu_e", bufs=2)
        nc.vector.tensor_scalar_min(m_t[:], ps, 0.0)
        nc.scalar.activation(e_t[:], m_t[:], AF.Exp)
        nc.vector.tensor_scalar_add(e_t[:], e_t[:], -1.0)
        if out_f32 is not None:
            nc.vector.tensor_tensor(out_f32[:, off:off + w], e_t[:], ps, ALU.max)
        if out_alt is not None:
            nc.vector.tensor_tensor(out_alt[:, off:off + w], e_t[:], ps, ALU.max)


def _build():
    nc = bacc.Bacc("TRN2", target_bir_lowering=False, debug=False,
                   num_devices=NCORES)
    rg = [list(range(NCORES))]

    adjT = nc.dram_tensor("adjT", [N, SH], BF16, kind="ExternalInput")
    xT = nc.dram_tensor("xT", [NFEAT, SH], BF16, kind="ExternalInput")
    w0 = nc.dram_tensor("w0", [NFEAT, F], BF16, kind="ExternalInput")
    w1 = nc.dram_tensor("w1", [F, F], BF16, kind="ExternalInput")
    lw = nc.dram_tensor("lw", [F + 1, NCLASS], BF16, kind="ExternalInput")

    emb0T_d = nc.dram_tensor("emb0T", [F, SH], F32, kind="ExternalOutput")
    emb1T_d = nc.dram_tensor("emb1T", [F, SH], F32, kind="ExternalOutput")
    outp_d = nc.dram_tensor("outp", [SH, NCLASS], F32, kind="ExternalOutput")
    ls_d = nc.dram_tensor("ls", [SH, NCLASS], F32, kind="ExternalOutput")

    with tile.TileContext(nc) as tc:
        with tc.tile_pool(name="sb", bufs=1) as sb, \
             tc.tile_pool(name="scratch", bufs=2) as scratch, \
             tc.tile_pool(name="psum_acc", bufs=2, space="PSUM") as pacc, \
             tc.tile_pool(name="psum_sm", bufs=3, space="PSUM") as psm, \
             tc.tile_pool(name="dram", bufs=1, space="DRAM") as dram:

            # ---- persistent SBUF tensors ----
            adjT_sb = sb.tile([128, NCH, SH], BF16, name="adjT_sb")
            xT_sb = sb.tile([128, XCH, SH], BF16, name="xT_sb")
            w0_sb = sb.tile([128, XCH, F], BF16, name="w0_sb")
            w1_sb = sb.tile([F, F], BF16, name="w1_sb")
            lw_sb = sb.tile([F + 1, NCLASS], BF16, name="lw_sb")
            h0_sb = sb.tile([128, NCH, F], BF16, name="h0_sb")
            h1_sb = sb.tile([128, NCH, F], BF16, name="h1_sb")
            emb0T = sb.tile([F, SH], F32, name="emb0T")
            emb0Tb = sb.tile([F, SH], BF16, name="emb0Tb")
            emb1T = sb.tile([F, SH], F32, name="emb1T")
            e2T = sb.tile([F + 1, SH], BF16, name="e2T")
            h0loc = sb.tile([128, SHC, F], BF16, name="h0loc")
            h1loc = sb.tile([128, SHC, F], BF16, name="h1loc")
            out_sb = sb.tile([128, SHC, NCLASS], F32, name="out_sb")
            t_sb = sb.tile([128, SHC, NCLASS], F32, name="t_sb")
            ls_sb = sb.tile([128, SHC, NCLASS], F32, name="ls_sb")
            mx_sb = sb.tile([128, SHC], F32, name="mx_sb")
            s_sb = sb.tile([128, SHC], F32, name="s_sb")
            l_sb = sb.tile([128, SHC], F32, name="l_sb")

            # ---- input DMAs ----
            adjT_r = adjT.ap().rearrange("(c p) n -> p c n", p=128)
            gsz = NCH // ADJ_DMA_GROUPS
            for g in range(ADJ_DMA_GROUPS):
                nc.sync.dma_start(
                    out=adjT_sb[:, g * gsz:(g + 1) * gsz, :],
                    in_=adjT_r[:, g * gsz:(g + 1) * gsz, :])
            nc.sync.dma_start(out=xT_sb[:],
                              in_=xT.ap().rearrange("(c p) n -> p c n", p=128))
            nc.sync.dma_start(out=w0_sb[:],
                              in_=w0.ap().rearrange("(c p) f -> p c f", p=128))
            nc.sync.dma_start(out=w1_sb[:], in_=w1[:])
            nc.sync.dma_start(out=lw_sb[:], in_=lw[:])
            nc.vector.memset(e2T[F:F + 1, :], 1.0)

            # ---- H0 = x @ W0cat (node-major tiles), bounce, AllGather ----
            for m in range(SHC):
                ph0 = psm.tile([128, F], F32, name=f"ph0_{m}", tag="psm")
                for kc in range(XCH):
                    nc.tensor.matmul(ph0[:], xT_sb[:, kc, m * 128:(m + 1) * 128],
                                     w0_sb[:, kc, :],
                                     start=(kc == 0), stop=(kc == XCH - 1))
                nc.scalar.activation(h0loc[:, m, :], ph0[:], AF.Copy)
            h0_bounce = dram.tile([SH, F], BF16, name="h0_bounce")
            h0_full = dram.tile([N, F], BF16, name="h0_full", addr_space="Shared")
            nc.sync.dma_start(
                out=h0_bounce.rearrange("(c p) f -> p c f", p=128),
                in_=h0loc[:])
            nc.gpsimd.collective_compute(
                "AllGather", ALU.bypass, replica_groups=rg,
                ins=[h0_bounce[:]], outs=[h0_full[:]])
            nc.sync.dma_start(out=h0_sb[:],
                              in_=h0_full.rearrange("(c p) f -> p c f", p=128))

            # ---- layer 0 big matmul: P0^T[f, n] over 64 K-chunks ----
            p0a = pacc.tile([F, 512], F32, name="p0a", tag="acc")
            p0b = pacc.tile([F, 512], F32, name="p0b", tag="acc")
            for c in range(NCH):
                st, sp = (c == 0), (c == NCH - 1)
                nc.tensor.matmul(p0a[:], h0_sb[:, c, :], adjT_sb[:, c, 0:512],
                                 start=st, stop=sp)
                nc.tensor.matmul(p0b[:], h0_sb[:, c, :], adjT_sb[:, c, 512:SH],
                                 start=st, stop=sp)

            # ---- elu -> emb0T (f32 out) + bf16 copy ----
            _emit_elu(nc, scratch, emb0T, None,
                      [(p0a[:], 0), (p0b[:], 512)], "l0")
            nc.sync.dma_start(out=emb0T_d[:], in_=emb0T[:])
            nc.vector.tensor_copy(emb0Tb[:], emb0T[:])

            # ---- H1 tiles = emb0 @ W1cat (node-major), bounce, AllGather ----
            for m in range(SHC):
                ph1 = psm.tile([128, F], F32, name=f"ph1_{m}", tag="psm")
                nc.tensor.matmul(ph1[:], emb0Tb[:, m * 128:(m + 1) * 128],
                                 w1_sb[:], start=True, stop=True)
                nc.scalar.activation(h1loc[:, m, :], ph1[:], AF.Copy)
            h1_bounce = dram.tile([SH, F], BF16, name="h1_bounce")
            h1_full = dram.tile([N, F], BF16, name="h1_full", addr_space="Shared")
            nc.sync.dma_start(
                out=h1_bounce.rearrange("(c p) f -> p c f", p=128),
                in_=h1loc[:])
            nc.gpsimd.collective_compute(
                "AllGather", ALU.bypass, replica_groups=rg,
                ins=[h1_bounce[:]], outs=[h1_full[:]])
            nc.sync.dma_start(out=h1_sb[:],
                              in_=h1_full.rearrange("(c p) f -> p c f", p=128))

            # ---- layer 1 big matmul ----
            p1a = pacc.tile([F, 512], F32, name="p1a", tag="acc")
            p1b = pacc.tile([F, 512], F32, name="p1b", tag="acc")
            for c in range(NCH):
                st, sp = (c == 0), (c == NCH - 1)
                nc.tensor.matmul(p1a[:], h1_sb[:, c, :], adjT_sb[:, c, 0:512],
                                 start=st, stop=sp)
                nc.tensor.matmul(p1b[:], h1_sb[:, c, :], adjT_sb[:, c, 512:SH],
                                 start=st, stop=sp)

            # ---- elu -> emb1T f32 ----
            _emit_elu(nc, scratch, emb1T, None,
                      [(p1a[:], 0), (p1b[:], 512)], "l1")
            nc.sync.dma_start(out=emb1T_d[:], in_=emb1T[:])

            # ---- e2 = elu(emb1) (bf16, with ones row for bias) ----
            _emit_elu(nc, scratch, None, e2T[0:F, :],
                      [(emb1T[:, 0:512], 0), (emb1T[:, 512:SH], 512)], "e2")

            # ---- classifier: out = e2 @ [lin_w; lin_b] (node-major) ----
            for m in range(SHC):
                pcls = psm.tile([128, NCLASS], F32, name=f"pcls_{m}", tag="psm")
                nc.tensor.matmul(pcls[:], e2T[:, m * 128:(m + 1) * 128],
                                 lw_sb[:], start=True, stop=True)
                nc.scalar.activation(out_sb[:, m, :], pcls[:], AF.Copy)
            nc.sync.dma_start(
                out=outp_d.ap().rearrange("(c p) f -> p c f", p=128),
                in_=out_sb[:])

            # ---- log_softmax over classes (free axis) ----
            for m in range(SHC):
                nc.vector.tensor_reduce(mx_sb[:, m:m + 1], out_sb[:, m, :],
                                        mybir.AxisListType.X, ALU.max)
                nc.vector.tensor_scalar(t_sb[:, m, :], out_sb[:, m, :],
                                        mx_sb[:, m:m + 1], None, ALU.subtract)
                e_sm = scratch.tile([128, NCLASS], F32, name=f"e_sm_{m}",
                                    tag="e_sm", bufs=2)
                nc.scalar.activation(e_sm[:], t_sb[:, m, :], AF.Exp,
                                     accum_out=s_sb[:, m:m + 1])
            nc.scalar.activation(l_sb[:], s_sb[:], AF.Ln)
            for m in range(SHC):
                nc.vector.tensor_scalar(ls_sb[:, m, :], t_sb[:, m, :],
                                        l_sb[:, m:m + 1], None, ALU.subtract)
            nc.sync.dma_start(
                out=ls_d.ap().rearrange("(c p) f -> p c f", p=128),
                in_=ls_sb[:])

    nc.compile()
    return nc


def _get_nc():
    if "nc" not in _NC_CACHE:
        _NC_CACHE["nc"] = _build()
    return _NC_CACHE["nc"]


def _prep_inputs(x, adj, W0, W1, lin_w, lin_b):
    bf = ml_dtypes.bfloat16
    w0cat = np.transpose(np.asarray(W0, np.float32), (1, 0, 2)).reshape(NFEAT, F)
    w1cat = np.transpose(np.asarray(W1, np.float32), (1, 0, 2)).reshape(F, F)
    lw_aug = np.concatenate(
        [np.asarray(lin_w, np.float32),
         np.asarray(lin_b, np.float32)[None, :]], axis=0)
    w0_b = w0cat.astype(bf)
    w1_b = w1cat.astype(bf)
    lw_b = lw_aug.astype(bf)
    xT = np.ascontiguousarray(np.asarray(x, np.float32).T).astype(bf)
    adj32 = np.asarray(adj, np.float32)
    in_maps = []
    for i in range(NCORES):
        sl = slice(i * SH, (i + 1) * SH)
        in_maps.append({
            "adjT": np.ascontiguousarray(adj32[sl, :].T).astype(bf),
            "xT": np.ascontiguousarray(xT[:, sl]),
            "w0": w0_b, "w1": w1_b, "lw": lw_b,
        })
    return in_maps


def kernel(x, adj, W0, a0, W1, a1, lin_w, lin_b):
    nc = _get_nc()
    in_maps = _prep_inputs(x, adj, W0, W1, lin_w, lin_b)
    res = run_bass_kernel_spmd(nc, in_maps, list(range(NCORES)))
    emb0 = np.concatenate([r["emb0T"].T for r in res.results], axis=0)
    emb1 = np.concatenate([r["emb1T"].T for r in res.results], axis=0)
    outp = np.concatenate([r["outp"] for r in res.results], axis=0)
    ls = np.concatenate([r["ls"] for r in res.results], axis=0)
    return (np.ascontiguousarray(ls), np.ascontiguousarray(emb0),
            np.ascontiguousarray(emb1), np.ascontiguousarray(outp))


# revision 7
# speedup vs baseline: 33.5263x; 33.5263x over previous
"""GAT (nn_GAT_76536317214930) on 8 TRN2 NeuronCores.

The reference's attention softmax is dead code; each layer is
    emb = elu(adj @ (x @ Wcat))        with heads concatenated on features,
then out = elu(emb1) @ lin_w + lin_b and log_softmax.

Sharding: rows (destination nodes) of adj split across 8 cores. Each core
holds adjT shard [8192, 1024] (bf16, SBUF-resident, used by both layers),
computes H = x @ Wcat for its own rows, AllGathers H, then accumulates
P^T = H^T-chunks (stationary) @ adjT-chunks (moving) on the PE.

All feature-major [64, n] intermediates; node-major tiles are produced by
matmuls with the feature-major tensor as lhsT (contracting over features),
so no explicit transposes are needed anywhere.
"""
import numpy as np
import ml_dtypes

import jax
from jax.experimental.shard_map import shard_map
from jax.sharding import Mesh, NamedSharding, PartitionSpec

import concourse.bass as bass
import concourse.bacc as bacc
import concourse.mybir as mybir
import concourse.tile as tile
from concourse import bass2jax

NCORES = 8
N = 8192          # nodes
NFEAT = 512       # input features
F = 64            # NHEADS * NHID = 4*16
NCLASS = 40
SH = N // NCORES  # 1024 rows per core
NCH = N // 128    # 64 contraction chunks (global nodes)
SHC = SH // 128   # 8 node tiles per core shard
XCH = NFEAT // 128  # 4 chunks of input features

BF16 = mybir.dt.bfloat16
F32 = mybir.dt.float32
AF = mybir.ActivationFunctionType
ALU = mybir.AluOpType

ADJ_DMA_GROUPS = 16  # adjT loaded in 16 DMAs of 4 chunks (1 MiB each)

_NC_CACHE = {}


def _emit_elu(nc, pool, out_f32, out_alt, src_psum_halves, tmp_tag):
    """elu(x) = max(x, exp(min(x, 0)) - 1), per [64, 512] half.

    src_psum_halves: list of (psum_ap, col_offset) covering [64, 1024].
    out_f32: [64, 1024] f32 SBUF AP or None.
    out_alt: (ap, dtype-done-by-ap) secondary output [64, 1024] or None.
    """
    for ps, off in src_psum_halves:
        w = ps.shape[-1]
        m_t = pool.tile([64, w], F32, name=f"{tmp_tag}_m", tag="elu_m", bufs=2)
        e_t = pool.tile([64, w], F32, name=f"{tmp_tag}_e", tag="elu_e", bufs=2)
        nc.vector.tensor_scalar_min(m_t[:], ps, 0.0)
        nc.scalar.activation(e_t[:], m_t[:], AF.Exp)
        nc.vector.tensor_scalar_add(e_t[:], e_t[:], -1.0)
        if out_f32 is not None:
            nc.vector.tensor_tensor(out_f32[:, off:off + w], e_t[:], ps, ALU.max)
        if out_alt is not None:
            nc.vector.tensor_tensor(out_alt[:, off:off + w], e_t[:], ps, ALU.max)


def _build():
    nc = bacc.Bacc("TRN2", target_bir_lowering=False, debug=False,
                   num_devices=NCORES)
    rg = [list(range(NCORES))]

    adjT = nc.dram_tensor("adjT", [N, SH], BF16, kind="ExternalInput")
    xT = nc.dram_tensor("xT", [NFEAT, SH], BF16, kind="ExternalInput")
    w0 = nc.dram_tensor("w0", [NFEAT, F], BF16, kind="ExternalInput")
    w1 = nc.dram_tensor("w1", [F, F], BF16, kind="ExternalInput")
    lw = nc.dram_tensor("lw", [F + 1, NCLASS], BF16, kind="ExternalInput")

    emb0T_d = nc.dram_tensor("emb0T", [F, SH], F32, kind="ExternalOutput")
    emb1T_d = nc.dram_tensor("emb1T", [F, SH], F32, kind="ExternalOutput")
    outp_d = nc.dram_tensor("outp", [SH, NCLASS], F32, kind="ExternalOutput")
    ls_d = nc.dram_tensor("ls", [SH, NCLASS], F32, kind="ExternalOutput")

    with tile.TileContext(nc) as tc:
        with tc.tile_pool(name="sb", bufs=1) as sb, \
             tc.tile_pool(name="scratch", bufs=2) as scratch, \
             tc.tile_pool(name="psum_acc", bufs=2, space="PSUM") as pacc, \
             tc.tile_pool(name="psum_sm", bufs=3, space="PSUM") as psm, \
             tc.tile_pool(name="dram", bufs=1, space="DRAM") as dram:

            # ---- persistent SBUF tensors ----
            adjT_sb = sb.tile([128, NCH, SH], BF16, name="adjT_sb")
            xT_sb = sb.tile([128, XCH, SH], BF16, name="xT_sb")
            w0_sb = sb.tile([128, XCH, F], BF16, name="w0_sb")
            w1_sb = sb.tile([F, F], BF16, name="w1_sb")
            lw_sb = sb.tile([F + 1, NCLASS], BF16, name="lw_sb")
            h0_sb = sb.tile([128, NCH, F], BF16, name="h0_sb")
            h1_sb = sb.tile([128, NCH, F], BF16, name="h1_sb")
            emb0T = sb.tile([F, SH], F32, name="emb0T")
            emb0Tb = sb.tile([F, SH], BF16, name="emb0Tb")
            emb1T = sb.tile([F, SH], F32, name="emb1T")
            e2T = sb.tile([F + 1, SH], BF16, name="e2T")
            h0loc = sb.tile([128, SHC, F], BF16, name="h0loc")
            h1loc = sb.tile([128, SHC, F], BF16, name="h1loc")
            out_sb = sb.tile([128, SHC, NCLASS], F32, name="out_sb")
            t_sb = sb.tile([128, SHC, NCLASS], F32, name="t_sb")
            ls_sb = sb.tile([128, SHC, NCLASS], F32, name="ls_sb")
            mx_sb = sb.tile([128, SHC], F32, name="mx_sb")
            s_sb = sb.tile([128, SHC], F32, name="s_sb")
            l_sb = sb.tile([128, SHC], F32, name="l_sb")

            # ---- input DMAs ----
            adjT_r = adjT.ap().rearrange("(c p) n -> p c n", p=128)
            gsz = NCH // ADJ_DMA_GROUPS
            for g in range(ADJ_DMA_GROUPS):
                nc.sync.dma_start(
                    out=adjT_sb[:, g * gsz:(g + 1) * gsz, :],
                    in_=adjT_r[:, g * gsz:(g + 1) * gsz, :])
            nc.sync.dma_start(out=xT_sb[:],
                              in_=xT.ap().rearrange("(c p) n -> p c n", p=128))
            nc.sync.dma_start(out=w0_sb[:],
                              in_=w0.ap().rearrange("(c p) f -> p c f", p=128))
            nc.sync.dma_start(out=w1_sb[:], in_=w1[:])
            nc.sync.dma_start(out=lw_sb[:], in_=lw[:])
            nc.vector.memset(e2T[F:F + 1, :], 1.0)

            # ---- H0 = x @ W0cat (node-major tiles), bounce, AllGather ----
            for m in range(SHC):
                ph0 = psm.tile([128, F], F32, name=f"ph0_{m}", tag="psm")
                for kc in range(XCH):
                    nc.tensor.matmul(ph0[:], xT_sb[:, kc, m * 128:(m + 1) * 128],
                                     w0_sb[:, kc, :],
                                     start=(kc == 0), stop=(kc == XCH - 1))
                nc.scalar.activation(h0loc[:, m, :], ph0[:], AF.Copy)
            h0_bounce = dram.tile([SH, F], BF16, name="h0_bounce")
            h0_full = dram.tile([N, F], BF16, name="h0_full", addr_space="Shared")
            nc.sync.dma_start(
                out=h0_bounce.rearrange("(c p) f -> p c f", p=128),
                in_=h0loc[:])
            nc.gpsimd.collective_compute(
                "AllGather", ALU.bypass, replica_groups=rg,
                ins=[h0_bounce[:]], outs=[h0_full[:]])
            nc.sync.dma_start(out=h0_sb[:],
                              in_=h0_full.rearrange("(c p) f -> p c f", p=128))

            # ---- layer 0 big matmul: P0^T[f, n] over 64 K-chunks ----
            p0a = pacc.tile([F, 512], F32, name="p0a", tag="acc")
            p0b = pacc.tile([F, 512], F32, name="p0b", tag="acc")
            for c in range(NCH):
                st, sp = (c == 0), (c == NCH - 1)
                nc.tensor.matmul(p0a[:], h0_sb[:, c, :], adjT_sb[:, c, 0:512],
                                 start=st, stop=sp)
                nc.tensor.matmul(p0b[:], h0_sb[:, c, :], adjT_sb[:, c, 512:SH],
                                 start=st, stop=sp)

            # ---- elu -> emb0T (f32 out) + bf16 copy ----
            _emit_elu(nc, scratch, emb0T, None,
                      [(p0a[:], 0), (p0b[:], 512)], "l0")
            nc.sync.dma_start(out=emb0T_d[:], in_=emb0T[:])
            nc.vector.tensor_copy(emb0Tb[:], emb0T[:])

            # ---- H1 tiles = emb0 @ W1cat (node-major), bounce, AllGather ----
            for m in range(SHC):
                ph1 = psm.tile([128, F], F32, name=f"ph1_{m}", tag="psm")
                nc.tensor.matmul(ph1[:], emb0Tb[:, m * 128:(m + 1) * 128],
                                 w1_sb[:], start=True, stop=True)
                nc.scalar.activation(h1loc[:, m, :], ph1[:], AF.Copy)
            h1_bounce = dram.tile([SH, F], BF16, name="h1_bounce")
            h1_full = dram.tile([N, F], BF16, name="h1_full", addr_space="Shared")
            nc.sync.dma_start(
                out=h1_bounce.rearrange("(c p) f -> p c f", p=128),
                in_=h1loc[:])
            nc.gpsimd.collective_compute(
                "AllGather", ALU.bypass, replica_groups=rg,
                ins=[h1_bounce[:]], outs=[h1_full[:]])
            nc.sync.dma_start(out=h1_sb[:],
                              in_=h1_full.rearrange("(c p) f -> p c f", p=128))

            # ---- layer 1 big matmul ----
            p1a = pacc.tile([F, 512], F32, name="p1a", tag="acc")
            p1b = pacc.tile([F, 512], F32, name="p1b", tag="acc")
            for c in range(NCH):
                st, sp = (c == 0), (c == NCH - 1)
                nc.tensor.matmul(p1a[:], h1_sb[:, c, :], adjT_sb[:, c, 0:512],
                                 start=st, stop=sp)
                nc.tensor.matmul(p1b[:], h1_sb[:, c, :], adjT_sb[:, c, 512:SH],
                                 start=st, stop=sp)

            # ---- elu -> emb1T f32 ----
            _emit_elu(nc, scratch, emb1T, None,
                      [(p1a[:], 0), (p1b[:], 512)], "l1")
            nc.sync.dma_start(out=emb1T_d[:], in_=emb1T[:])

            # ---- e2 = elu(emb1) (bf16, with ones row for bias) ----
            _emit_elu(nc, scratch, None, e2T[0:F, :],
                      [(emb1T[:, 0:512], 0), (emb1T[:, 512:SH], 512)], "e2")

            # ---- classifier: out = e2 @ [lin_w; lin_b] (node-major) ----
            for m in range(SHC):
                pcls = psm.tile([128, NCLASS], F32, name=f"pcls_{m}", tag="psm")
                nc.tensor.matmul(pcls[:], e2T[:, m * 128:(m + 1) * 128],
                                 lw_sb[:], start=True, stop=True)
                nc.scalar.activation(out_sb[:, m, :], pcls[:], AF.Copy)
            nc.sync.dma_start(
                out=outp_d.ap().rearrange("(c p) f -> p c f", p=128),
                in_=out_sb[:])

            # ---- log_softmax over classes (free axis) ----
            for m in range(SHC):
                nc.vector.tensor_reduce(mx_sb[:, m:m + 1], out_sb[:, m, :],
                                        mybir.AxisListType.X, ALU.max)
                nc.vector.tensor_scalar(t_sb[:, m, :], out_sb[:, m, :],
                                        mx_sb[:, m:m + 1], None, ALU.subtract)
                e_sm = scratch.tile([128, NCLASS], F32, name=f"e_sm_{m}",
                                    tag="e_sm", bufs=2)
                nc.scalar.activation(e_sm[:], t_sb[:, m, :], AF.Exp,
                                     accum_out=s_sb[:, m:m + 1])
            nc.scalar.activation(l_sb[:], s_sb[:], AF.Ln)
            for m in range(SHC):
                nc.vector.tensor_scalar(ls_sb[:, m, :], t_sb[:, m, :],
                                        l_sb[:, m:m + 1], None, ALU.subtract)
            nc.sync.dma_start(
                out=ls_d.ap().rearrange("(c p) f -> p c f", p=128),
                in_=ls_sb[:])

    nc.compile()
    return nc


def _get_nc():
    if "nc" not in _NC_CACHE:
        _NC_CACHE["nc"] = _build()
    return _NC_CACHE["nc"]


class _Runner:
    """One-time jit of the SPMD NEFF executable; repeat calls just execute.

    Mirrors bass2jax.run_bass_via_pjrt's multi-core path, minus donation,
    so device-resident inputs can be reused across timed calls.
    """

    def __init__(self, nc):
        bass2jax.install_neuronx_cc_hook()
        self.nc = nc
        partition_name = (nc.partition_id_tensor.name
                          if nc.partition_id_tensor else None)
        in_names, out_names, out_avals, zero_outs = [], [], [], []
        for alloc in nc.m.functions[0].allocations:
            if not isinstance(alloc, mybir.MemoryLocationSet):
                continue
            name = alloc.memorylocations[0].name
            if alloc.kind == "ExternalInput":
                if name != partition_name:
                    in_names.append(name)
            elif alloc.kind == "ExternalOutput":
                shape = tuple(alloc.tensor_shape)
                dtype = mybir.dt.np(alloc.dtype)
                out_names.append(name)
                out_avals.append(jax.core.ShapedArray(shape, dtype))
                zero_outs.append(np.zeros(shape, dtype))
        self.n_params = len(in_names)
        self.in_names = list(in_names)
        self.out_names = out_names
        self.out_avals = out_avals
        all_in_names = list(in_names) + list(out_names)
        if partition_name is not None:
            all_in_names.append(partition_name)

        def _body(*args):
            operands = list(args)
            if partition_name is not None:
                operands.append(bass2jax.partition_id_tensor())
            outs = bass2jax._bass_exec_p.bind(
                *operands,
                out_avals=tuple(out_avals),
                in_names=tuple(all_in_names),
                out_names=tuple(out_names),
                lowering_input_output_aliases=(),
                sim_require_finite=True,
                sim_require_nnan=True,
                nc=nc,
            )
            return tuple(outs)

        devices = jax.devices()[:NCORES]
        self.mesh = Mesh(np.asarray(devices), ("core",))
        nspec = self.n_params + len(out_names)
        self.sharding = NamedSharding(self.mesh, PartitionSpec("core"))
        self.fn = jax.jit(
            shard_map(_body, mesh=self.mesh,
                      in_specs=(PartitionSpec("core"),) * nspec,
                      out_specs=(PartitionSpec("core"),) * len(out_names),
                      check_rep=False),
            keep_unused=True,
        )
        self.zero_dev = [
            jax.device_put(
                np.zeros((NCORES * z.shape[0], *z.shape[1:]), z.dtype),
                self.sharding)
            for z in zero_outs
        ]

    def put_inputs(self, in_maps):
        concat = [
            np.concatenate([np.asarray(in_maps[c][name])
                            for c in range(NCORES)], axis=0)
            for name in self.in_names
        ]
        return [jax.device_put(a, self.sharding) for a in concat]

    def execute(self, dev_inputs):
        outs = self.fn(*dev_inputs, *self.zero_dev)
        jax.block_until_ready(outs)
        return outs

    def __call__(self, in_maps):
        outs = self.execute(self.put_inputs(in_maps))
        res = []
        for c in range(NCORES):
            res.append({
                name: np.asarray(outs[i]).reshape(
                    NCORES, *self.out_avals[i].shape)[c]
                for i, name in enumerate(self.out_names)
            })
        return res


def _get_runner():
    if "runner" not in _NC_CACHE:
        _NC_CACHE["runner"] = _Runner(_get_nc())
    return _NC_CACHE["runner"]


def _prep_inputs(x, adj, W0, W1, lin_w, lin_b):
    bf = ml_dtypes.bfloat16
    w0cat = np.transpose(np.asarray(W0, np.float32), (1, 0, 2)).reshape(NFEAT, F)
    w1cat = np.transpose(np.asarray(W1, np.float32), (1, 0, 2)).reshape(F, F)
    lw_aug = np.concatenate(
        [np.asarray(lin_w, np.float32),
         np.asarray(lin_b, np.float32)[None, :]], axis=0)
    w0_b = w0cat.astype(bf)
    w1_b = w1cat.astype(bf)
    lw_b = lw_aug.astype(bf)
    xT = np.ascontiguousarray(np.asarray(x, np.float32).T).astype(bf)
    adj32 = np.asarray(adj, np.float32)
    in_maps = []
    for i in range(NCORES):
        sl = slice(i * SH, (i + 1) * SH)
        in_maps.append({
            "adjT": np.ascontiguousarray(adj32[sl, :].T).astype(bf),
            "xT": np.ascontiguousarray(xT[:, sl]),
            "w0": w0_b, "w1": w1_b, "lw": lw_b,
        })
    return in_maps


def kernel(x, adj, W0, a0, W1, a1, lin_w, lin_b):
    runner = _get_runner()
    in_maps = _prep_inputs(x, adj, W0, W1, lin_w, lin_b)
    results = runner(in_maps)
    emb0 = np.concatenate([r["emb0T"].T for r in results], axis=0)
    emb1 = np.concatenate([r["emb1T"].T for r in results], axis=0)
    outp = np.concatenate([r["outp"] for r in results], axis=0)
    ls = np.concatenate([r["ls"] for r in results], axis=0)
    return (np.ascontiguousarray(ls), np.ascontiguousarray(emb0),
            np.ascontiguousarray(emb1), np.ascontiguousarray(outp))


# revision 11
# speedup vs baseline: 478.7836x; 14.2808x over previous
"""GAT (nn_GAT_76536317214930) on 8 TRN2 NeuronCores.

The reference's attention softmax is dead code; each layer is
    emb = elu(adj @ (x @ Wcat))        with heads concatenated on features,
then out = elu(emb1) @ lin_w + lin_b and log_softmax.

Sharding: rows (destination nodes) of adj split across 8 cores. Each core
holds adjT shard [8192, 1024] (bf16, SBUF-resident, used by both layers),
computes H = x @ Wcat for its own rows, AllGathers H, then accumulates
P^T = H^T-chunks (stationary) @ adjT-chunks (moving) on the PE.

All feature-major [64, n] intermediates; node-major tiles are produced by
matmuls with the feature-major tensor as lhsT (contracting over features),
so no explicit transposes are needed anywhere.
"""
import numpy as np
import ml_dtypes

import jax
from jax.experimental.shard_map import shard_map
from jax.sharding import Mesh, NamedSharding, PartitionSpec

import concourse.bass as bass
import concourse.bacc as bacc
import concourse.mybir as mybir
import concourse.tile as tile
from concourse import bass2jax

NCORES = 8
N = 8192          # nodes
NFEAT = 512       # input features
F = 64            # NHEADS * NHID = 4*16
NCLASS = 40
SH = N // NCORES  # 1024 rows per core
NCH = N // 128    # 64 contraction chunks (global nodes)
SHC = SH // 128   # 8 node tiles per core shard
XCH = NFEAT // 128  # 4 chunks of input features

BF16 = mybir.dt.bfloat16
F32 = mybir.dt.float32
AF = mybir.ActivationFunctionType
ALU = mybir.AluOpType

ADJ_DMA_GROUPS = 16  # adjT loaded in 16 DMAs of 4 chunks (1 MiB each)

_NC_CACHE = {}


def _emit_elu(nc, pool, out_f32, out_alt, src_halves, tag):
    """elu(x) = max(x, exp(min(x, 0)) - 1), per [64, 512] half."""
    for i, (ps, off) in enumerate(src_halves):
        w = ps.shape[-1]
        m_t = pool.tile([F, w], F32, name=f"{tag}_m{i}", tag="elu_m", bufs=2)
        e_t = pool.tile([F, w], F32, name=f"{tag}_e{i}", tag="elu_e", bufs=2)
        nc.vector.tensor_scalar_min(m_t[:], ps, 0.0)
        nc.scalar.activation(e_t[:], m_t[:], AF.Exp)
        nc.vector.tensor_scalar_add(e_t[:], e_t[:], -1.0)
        if out_f32 is not None:
            nc.vector.tensor_tensor(out_f32[:, off:off + w], e_t[:], ps, ALU.max)
        if out_alt is not None:
            nc.vector.tensor_tensor(out_alt[:, off:off + w], e_t[:], ps, ALU.max)


def _emit_body(nc, tc, sb, scratch, pacc, psm, dram, io, rep):
    """One full forward pass. All tiles tagged so reps share SBUF slots."""
    rg = [list(range(NCORES))]
    adjT, xT, w0, w1, lw, emb0T_d, emb1T_d, outp_d, ls_d = io
    r = f"_{rep}"

    def T(pool, shape, dtype, nm, **kw):
        return pool.tile(shape, dtype, name=nm + r, tag=nm, **kw)

    # ---- persistent SBUF tensors ----
    adjT_sb = T(sb, [128, NCH, SH], BF16, "adjT_sb")
    xT_sb = T(sb, [128, XCH, SH], BF16, "xT_sb")
    w0_sb = T(sb, [128, XCH, F], BF16, "w0_sb")
    w1_sb = T(sb, [F, F], BF16, "w1_sb")
    lw_sb = T(sb, [F + 1, NCLASS], BF16, "lw_sb")
    h0_sb = T(sb, [128, NCH, F], BF16, "h0_sb")
    h1_sb = T(sb, [128, NCH, F], BF16, "h1_sb")
    emb0T = T(sb, [F, SH], F32, "emb0T")
    emb0Tb = T(sb, [F, SH], BF16, "emb0Tb")
    emb1T = T(sb, [F, SH], F32, "emb1T")
    e2T = T(sb, [F + 1, SH], BF16, "e2T")
    h0loc = T(sb, [128, SHC, F], BF16, "h0loc")
    h1loc = T(sb, [128, SHC, F], BF16, "h1loc")
    out_sb = T(sb, [128, SHC, NCLASS], F32, "out_sb")
    t_sb = T(sb, [128, SHC, NCLASS], F32, "t_sb")
    ls_sb = T(sb, [128, SHC, NCLASS], F32, "ls_sb")
    mx_sb = T(sb, [128, SHC], F32, "mx_sb")
    s_sb = T(sb, [128, SHC], F32, "s_sb")
    l_sb = T(sb, [128, SHC], F32, "l_sb")

    # ---- input DMAs ----
    adjT_r = adjT.ap().rearrange("(c p) n -> p c n", p=128)
    gsz = NCH // ADJ_DMA_GROUPS
    for g in range(ADJ_DMA_GROUPS):
        nc.sync.dma_start(
            out=adjT_sb[:, g * gsz:(g + 1) * gsz, :],
            in_=adjT_r[:, g * gsz:(g + 1) * gsz, :])
    nc.sync.dma_start(out=xT_sb[:],
                      in_=xT.ap().rearrange("(c p) n -> p c n", p=128))
    nc.sync.dma_start(out=w0_sb[:],
                      in_=w0.ap().rearrange("(c p) f -> p c f", p=128))
    nc.sync.dma_start(out=w1_sb[:], in_=w1[:])
    nc.sync.dma_start(out=lw_sb[:], in_=lw[:])
    nc.vector.memset(e2T[F:F + 1, :], 1.0)

    # ---- H0 = x @ W0cat (node-major tiles), bounce, AllGather ----
    for m in range(SHC):
        ph0 = psm.tile([128, F], F32, name=f"ph0_{m}{r}", tag="psm")
        for kc in range(XCH):
            nc.tensor.matmul(ph0[:], xT_sb[:, kc, m * 128:(m + 1) * 128],
                             w0_sb[:, kc, :],
                             start=(kc == 0), stop=(kc == XCH - 1))
        nc.scalar.activation(h0loc[:, m, :], ph0[:], AF.Copy)
    h0_bounce = T(dram, [SH, F], BF16, "h0_bounce")
    h0_full = T(dram, [N, F], BF16, "h0_full", addr_space="Shared")
    nc.sync.dma_start(
        out=h0_bounce.rearrange("(c p) f -> p c f", p=128), in_=h0loc[:])
    nc.gpsimd.collective_compute(
        "AllGather", ALU.bypass, replica_groups=rg,
        ins=[h0_bounce[:]], outs=[h0_full[:]])
    nc.sync.dma_start(out=h0_sb[:],
                      in_=h0_full.rearrange("(c p) f -> p c f", p=128))

    # ---- layer 0 big matmul: P0^T[f, n] accumulated over 64 K-chunks ----
    p0a = T(pacc, [F, 512], F32, "acc_a")
    p0b = T(pacc, [F, 512], F32, "acc_b")
    for c in range(NCH):
        st, sp = (c == 0), (c == NCH - 1)
        nc.tensor.matmul(p0a[:], h0_sb[:, c, :], adjT_sb[:, c, 0:512],
                         start=st, stop=sp)
        nc.tensor.matmul(p0b[:], h0_sb[:, c, :], adjT_sb[:, c, 512:SH],
                         start=st, stop=sp)

    # ---- elu -> emb0T (f32 out) + bf16 copy ----
    _emit_elu(nc, scratch, emb0T, None, [(p0a[:], 0), (p0b[:], 512)], "l0" + r)
    nc.sync.dma_start(out=emb0T_d[:], in_=emb0T[:])
    nc.vector.tensor_copy(emb0Tb[:], emb0T[:])

    # ---- H1 tiles = emb0 @ W1cat (node-major), bounce, AllGather ----
    for m in range(SHC):
        ph1 = psm.tile([128, F], F32, name=f"ph1_{m}{r}", tag="psm")
        nc.tensor.matmul(ph1[:], emb0Tb[:, m * 128:(m + 1) * 128],
                         w1_sb[:], start=True, stop=True)
        nc.scalar.activation(h1loc[:, m, :], ph1[:], AF.Copy)
    h1_bounce = T(dram, [SH, F], BF16, "h1_bounce")
    h1_full = T(dram, [N, F], BF16, "h1_full", addr_space="Shared")
    nc.sync.dma_start(
        out=h1_bounce.rearrange("(c p) f -> p c f", p=128), in_=h1loc[:])
    nc.gpsimd.collective_compute(
        "AllGather", ALU.bypass, replica_groups=rg,
        ins=[h1_bounce[:]], outs=[h1_full[:]])
    nc.sync.dma_start(out=h1_sb[:],
                      in_=h1_full.rearrange("(c p) f -> p c f", p=128))

    # ---- layer 1 big matmul ----
    p1a = pacc.tile([F, 512], F32, name=f"acc_a2{r}", tag="acc_a")
    p1b = pacc.tile([F, 512], F32, name=f"acc_b2{r}", tag="acc_b")
    for c in range(NCH):
        st, sp = (c == 0), (c == NCH - 1)
        nc.tensor.matmul(p1a[:], h1_sb[:, c, :], adjT_sb[:, c, 0:512],
                         start=st, stop=sp)
        nc.tensor.matmul(p1b[:], h1_sb[:, c, :], adjT_sb[:, c, 512:SH],
                         start=st, stop=sp)

    # ---- elu -> emb1T f32; e2 = elu(emb1) bf16 (ones row = bias) ----
    _emit_elu(nc, scratch, emb1T, None, [(p1a[:], 0), (p1b[:], 512)], "l1" + r)
    nc.sync.dma_start(out=emb1T_d[:], in_=emb1T[:])
    _emit_elu(nc, scratch, None, e2T[0:F, :],
              [(emb1T[:, 0:512], 0), (emb1T[:, 512:SH], 512)], "e2" + r)

    # ---- classifier: out = e2 @ [lin_w; lin_b] (node-major) ----
    for m in range(SHC):
        pcls = psm.tile([128, NCLASS], F32, name=f"pcls_{m}{r}", tag="psm")
        nc.tensor.matmul(pcls[:], e2T[:, m * 128:(m + 1) * 128],
                         lw_sb[:], start=True, stop=True)
        nc.scalar.activation(out_sb[:, m, :], pcls[:], AF.Copy)
    nc.sync.dma_start(
        out=outp_d.ap().rearrange("(c p) f -> p c f", p=128), in_=out_sb[:])

    # ---- log_softmax over classes (free axis) ----
    for m in range(SHC):
        nc.vector.tensor_reduce(mx_sb[:, m:m + 1], out_sb[:, m, :],
                                mybir.AxisListType.X, ALU.max)
        nc.vector.tensor_scalar(t_sb[:, m, :], out_sb[:, m, :],
                                mx_sb[:, m:m + 1], None, ALU.subtract)
        e_sm = scratch.tile([128, NCLASS], F32, name=f"e_sm_{m}{r}",
                            tag="e_sm", bufs=2)
        nc.scalar.activation(e_sm[:], t_sb[:, m, :], AF.Exp,
                             accum_out=s_sb[:, m:m + 1])
    nc.scalar.activation(l_sb[:], s_sb[:], AF.Ln)
    for m in range(SHC):
        nc.vector.tensor_scalar(ls_sb[:, m, :], t_sb[:, m, :],
                                l_sb[:, m:m + 1], None, ALU.subtract)
    nc.sync.dma_start(
        out=ls_d.ap().rearrange("(c p) f -> p c f", p=128), in_=ls_sb[:])


def _build(reps=1):
    nc = bacc.Bacc("TRN2", target_bir_lowering=False, debug=False,
                   num_devices=NCORES)

    adjT = nc.dram_tensor("adjT", [N, SH], BF16, kind="ExternalInput")
    xT = nc.dram_tensor("xT", [NFEAT, SH], BF16, kind="ExternalInput")
    w0 = nc.dram_tensor("w0", [NFEAT, F], BF16, kind="ExternalInput")
    w1 = nc.dram_tensor("w1", [F, F], BF16, kind="ExternalInput")
    lw = nc.dram_tensor("lw", [F + 1, NCLASS], BF16, kind="ExternalInput")

    emb0T_d = nc.dram_tensor("emb0T", [F, SH], F32, kind="ExternalOutput")
    emb1T_d = nc.dram_tensor("emb1T", [F, SH], F32, kind="ExternalOutput")
    outp_d = nc.dram_tensor("outp", [SH, NCLASS], F32, kind="ExternalOutput")
    ls_d = nc.dram_tensor("ls", [SH, NCLASS], F32, kind="ExternalOutput")
    io = (adjT, xT, w0, w1, lw, emb0T_d, emb1T_d, outp_d, ls_d)

    with tile.TileContext(nc) as tc:
        with tc.tile_pool(name="sb", bufs=1) as sb, \
             tc.tile_pool(name="scratch", bufs=2) as scratch, \
             tc.tile_pool(name="psum_acc", bufs=2, space="PSUM") as pacc, \
             tc.tile_pool(name="psum_sm", bufs=3, space="PSUM") as psm, \
             tc.tile_pool(name="dram", bufs=1, space="DRAM") as dram:
            for rep in range(reps):
                if rep:
                    tc.strict_bb_all_engine_barrier()
                _emit_body(nc, tc, sb, scratch, pacc, psm, dram, io, rep)

    nc.compile()
    return nc


def _get_nc(reps=1):
    key = f"nc_{reps}"
    if key not in _NC_CACHE:
        _NC_CACHE[key] = _build(reps)
    return _NC_CACHE[key]


class _Runner:
    """One-time jit of the SPMD NEFF executable; repeat calls just execute.

    Mirrors bass2jax.run_bass_via_pjrt's multi-core path, minus donation,
    so device-resident inputs can be reused across timed calls.
    """

    def __init__(self, nc):
        bass2jax.install_neuronx_cc_hook()
        self.nc = nc
        partition_name = (nc.partition_id_tensor.name
                          if nc.partition_id_tensor else None)
        in_names, out_names, out_avals, zero_outs = [], [], [], []
        for alloc in nc.m.functions[0].allocations:
            if not isinstance(alloc, mybir.MemoryLocationSet):
                continue
            name = alloc.memorylocations[0].name
            if alloc.kind == "ExternalInput":
                if name != partition_name:
                    in_names.append(name)
            elif alloc.kind == "ExternalOutput":
                shape = tuple(alloc.tensor_shape)
                dtype = mybir.dt.np(alloc.dtype)
                out_names.append(name)
                out_avals.append(jax.core.ShapedArray(shape, dtype))
                zero_outs.append(np.zeros(shape, dtype))
        self.n_params = len(in_names)
        self.in_names = list(in_names)
        self.out_names = out_names
        self.out_avals = out_avals
        all_in_names = list(in_names) + list(out_names)
        if partition_name is not None:
            all_in_names.append(partition_name)

        def _body(*args):
            operands = list(args)
            if partition_name is not None:
                operands.append(bass2jax.partition_id_tensor())
            outs = bass2jax._bass_exec_p.bind(
                *operands,
                out_avals=tuple(out_avals),
                in_names=tuple(all_in_names),
                out_names=tuple(out_names),
                lowering_input_output_aliases=(),
                sim_require_finite=True,
                sim_require_nnan=True,
                nc=nc,
            )
            return tuple(outs)

        devices = jax.devices()[:NCORES]
        self.mesh = Mesh(np.asarray(devices), ("core",))
        nspec = self.n_params + len(out_names)
        self.sharding = NamedSharding(self.mesh, PartitionSpec("core"))
        self.fn = jax.jit(
            shard_map(_body, mesh=self.mesh,
                      in_specs=(PartitionSpec("core"),) * nspec,
                      out_specs=(PartitionSpec("core"),) * len(out_names),
                      check_rep=False),
            keep_unused=True,
        )
        self.zero_dev = [
            jax.device_put(
                np.zeros((NCORES * z.shape[0], *z.shape[1:]), z.dtype),
                self.sharding)
            for z in zero_outs
        ]

    def put_inputs(self, in_maps):
        concat = [
            np.concatenate([np.asarray(in_maps[c][name])
                            for c in range(NCORES)], axis=0)
            for name in self.in_names
        ]
        return [jax.device_put(a, self.sharding) for a in concat]

    def execute(self, dev_inputs):
        outs = self.fn(*dev_inputs, *self.zero_dev)
        jax.block_until_ready(outs)
        return outs

    def __call__(self, in_maps):
        outs = self.execute(self.put_inputs(in_maps))
        res = []
        for c in range(NCORES):
            res.append({
                name: np.asarray(outs[i]).reshape(
                    NCORES, *self.out_avals[i].shape)[c]
                for i, name in enumerate(self.out_names)
            })
        return res


def _get_runner(reps=1):
    key = f"runner_{reps}"
    if key not in _NC_CACHE:
        _NC_CACHE[key] = _Runner(_get_nc(reps))
    return _NC_CACHE[key]


def _prep_inputs(x, adj, W0, W1, lin_w, lin_b):
    bf = ml_dtypes.bfloat16
    w0cat = np.transpose(np.asarray(W0, np.float32), (1, 0, 2)).reshape(NFEAT, F)
    w1cat = np.transpose(np.asarray(W1, np.float32), (1, 0, 2)).reshape(F, F)
    lw_aug = np.concatenate(
        [np.asarray(lin_w, np.float32),
         np.asarray(lin_b, np.float32)[None, :]], axis=0)
    w0_b = w0cat.astype(bf)
    w1_b = w1cat.astype(bf)
    lw_b = lw_aug.astype(bf)
    xT = np.ascontiguousarray(np.asarray(x, np.float32).T).astype(bf)
    adj32 = np.asarray(adj, np.float32)
    in_maps = []
    for i in range(NCORES):
        sl = slice(i * SH, (i + 1) * SH)
        in_maps.append({
            "adjT": np.ascontiguousarray(adj32[sl, :].T).astype(bf),
            "xT": np.ascontiguousarray(xT[:, sl]),
            "w0": w0_b, "w1": w1_b, "lw": lw_b,
        })
    return in_maps


def kernel(x, adj, W0, a0, W1, a1, lin_w, lin_b):
    in_maps = _prep_inputs(x, adj, W0, W1, lin_w, lin_b)
    results = None
    for attempt in range(3):
        try:
            results = _get_runner()(in_maps)
            break
        except Exception:
            if attempt == 2:
                raise
            # Device may be wedged from a prior process; reset the PJRT
            # client and rebuild the jit (NEFF compile is cached).
            import jax._src.xla_bridge as _xb
            _NC_CACHE.pop("runner_1", None)
            try:
                _xb._clear_backends()
            except Exception:
                pass
    emb0 = np.concatenate([r["emb0T"].T for r in results], axis=0)
    emb1 = np.concatenate([r["emb1T"].T for r in results], axis=0)
    outp = np.concatenate([r["outp"] for r in results], axis=0)
    ls = np.concatenate([r["ls"] for r in results], axis=0)
    return (np.ascontiguousarray(ls), np.ascontiguousarray(emb0),
            np.ascontiguousarray(emb1), np.ascontiguousarray(outp))
